# revision 1
# baseline (speedup 1.0000x reference)
"""Trainium2 Bass kernel for nn_CMFA (dense_transformer, seq_len=1 cross-attention).

Math notes (exact simplifications vs the reference):
  - softmax over a single key is exactly 1.0, so the attention output is
    exactly the v-projection: mha(q,k,v) = (v @ Wv.T + bv) @ Wo.T + bo.
    The q/k projections never influence the output.
  - Wv -> Wo -> fi2 is a linear chain (no nonlinearity), so it is folded on
    the host:  V = [v1, i_] @ Wcat.T + bcat  with
      Wcat = [fi2 @ (Wo @ Wv), fi2],  bcat = fi2 @ (Wo @ bv + bo) + fi2_b
    (the i_ column block carries the residual through fi2).

Precision: matmul operands (inputs, weights, intermediate activations) are
fp16; PSUM accumulation, biases and the final output are fp32. fp16 keeps
the PE at 1 cycle/row (same as f32r at 512-wide moving dim) but halves the
stationary-weight load so it hides under the previous matmul's streaming
window (f32r pays ~25ns per matmul for it), and halves HBM traffic.
Measured end-to-end error vs the f32 reference is ~5e-4 (gate is 2e-2).

Device layout: activations are feature-major ("transposed", [feat, batch]) so
every matmul contracts over the partition dim and every DMA is contiguous.
The host pre-transposes the batch shards of i/t and transposes the output
back. Pure data parallel across 8 cores; weights replicated.

Schedule notes (all from trace measurements):
  - All loads ride the sync queue in program order: each DMA issue costs
    ~610ns of sequencer time, so tile-0 x / fi1-weight chunks are grouped
    (1/2/3/4/6) to stay ahead of the PE's 853ns-per-chunk consumption, and
    prefetches are issued behind the preamble so they cannot steal DMA
    bandwidth from startup-critical transfers.
  - Output stores issue from the scalar queue (hardware DGE, and the act
    that produces the tile runs there, so no cross-engine hop); the gpsimd
    queue's software DGE costs ~1us per store and serializes the tail.
  - The PE drops to half clock for ~3us after any idle gap, so a dozen
    warm-up matmuls on a memset tile bridge the DMA startup window and the
    real stream enters at full clock.
  - t-branch first (ft1 gates on only 192KB of input), then fi1 (k-outer:
    4 matmuls per arriving x chunk), then ct1/ci1 (hides the i_ activation
    latency), then the two folded output layers.
"""

import numpy as np

B, IMG, TAB, HID = 32768, 2048, 128, 512
NCORES = 8
BS = B // NCORES  # rows per core
NT = 512          # batch-tile (matmul moving/free dim)
KI = IMG // 128   # 16 contraction chunks for fi1
XALL = KI + 1     # + the t chunk, packed as chunk 0 of the same tile
NWARM = 9         # PE p-state warm-up matmuls (bridge ~4us of DMA latency)

_CACHE = {}


def _pack_blocks(WT: np.ndarray, K: int, M: int) -> np.ndarray:
    """[K*128, M*128] -> [128, K, M*128] with [p, k, m*128+j] = WT[k*128+p, m*128+j]."""
    out = WT.reshape(K, 128, M * 128).transpose(1, 0, 2)
    return np.ascontiguousarray(out, dtype=np.float16)


def _build_nc(bs: int):
    import concourse.bass as bass
    import concourse.tile as tile
    from concourse import bacc, mybir

    f32 = mybir.dt.float32
    f16 = mybir.dt.float16
    Relu = mybir.ActivationFunctionType.Relu
    Ident = mybir.ActivationFunctionType.Identity
    ntiles = bs // NT

    nc = bacc.Bacc("TRN2", target_bir_lowering=False, debug=False)

    # tile-major input layout: per batch-tile, all 17 chunks contiguous per
    # partition (16KB lines -> large DMA descriptors, one prefetch per tile)
    iT_d = nc.dram_tensor("iT", [bs // NT, 128, XALL, NT], f16,
                          kind="ExternalInput").ap()
    w_fi1_d = nc.dram_tensor("w_fi1", [128, KI, 512], f16, kind="ExternalInput").ap()
    w_ft1_d = nc.dram_tensor("w_ft1", [128, 1, 512], f16, kind="ExternalInput").ap()
    w_ci1_d = nc.dram_tensor("w_ci1", [128, 4, 512], f16, kind="ExternalInput").ap()
    w_ct1_d = nc.dram_tensor("w_ct1", [128, 4, 512], f16, kind="ExternalInput").ap()
    w_V_d = nc.dram_tensor("w_V", [128, 8, 512], f16, kind="ExternalInput").ap()
    w_T_d = nc.dram_tensor("w_T", [128, 8, 512], f16, kind="ExternalInput").ap()
    bias_d = nc.dram_tensor("bias", [128, 24], f32, kind="ExternalInput").ap()
    out_d = nc.dram_tensor("outT", [2 * HID, bs], f32, kind="ExternalOutput").ap()

    # fi1 chunk groups for tile 0 (i-chunk indices): sized so grouped DMA
    # issues stay ahead of the PE eating 4 matmuls (853ns) per chunk, finer
    # at the back where cumulative transfer time approaches the consumption
    # schedule. x tile chunk index = i-chunk + 1 (chunk 0 is the t chunk).
    WGRP = [(0, 1), (1, 3), (3, 6), (6, 9), (9, 12), (12, 14), (14, 16)]

    with tile.TileContext(nc) as tc:
        with (
            tc.tile_pool(name="w", bufs=1) as wpool,
            tc.tile_pool(name="x", bufs=3) as xpool,
            tc.tile_pool(name="h", bufs=8) as hpool,
            tc.tile_pool(name="o", bufs=8) as opool,
            tc.tile_pool(name="ps", bufs=8, space="PSUM") as pspool,
        ):
            wf1 = wpool.tile([128, KI, 512], f16, name="w_fi1_t")
            wt1 = wpool.tile([128, 1, 512], f16, name="w_ft1_t")
            wc1 = wpool.tile([128, 4, 512], f16, name="w_ci1_t")
            wc2 = wpool.tile([128, 4, 512], f16, name="w_ct1_t")
            wV = wpool.tile([128, 8, 512], f16, name="w_V_t")
            wT = wpool.tile([128, 8, 512], f16, name="w_T_t")
            bt = wpool.tile([128, 24], f32, name="bias_t")
            warm = wpool.tile([128, NT], f16, name="warm")

            # ---- PE warm-up: bridge the DMA startup window at rising clock ----
            nc.vector.memset(warm[:], 0.0)
            wps = pspool.tile([128, NT], f32, tag="ps", name="warm_ps")
            for _ in range(NWARM):
                nc.tensor.matmul(wps[:], warm[:, 0:128], warm[:], start=True, stop=True)

            # ---- preamble loads, earliest-deadline order, split across the
            # sync and scalar queues so issue cadence (~610ns each, in-order
            # per queue) never falls behind the PE's chunk consumption.
            # V/T of tile n are deferred into tile n+1, so wV/wT are not
            # needed until deep into tile 1, after the startup HBM crunch.
            x_cur = xpool.tile([128, XALL, NT], f16, tag="x", name="x_0")
            x_1 = xpool.tile([128, XALL, NT], f16, tag="x", name="x_1")
            nc.sync.dma_start(x_cur[:, 0, :], iT_d[0, :, 0, :])
            nc.sync.dma_start(wt1[:], w_ft1_d[:])
            nc.scalar.dma_start(bt[:], bias_d[:])
            for a, b in WGRP:
                nc.sync.dma_start(x_cur[:, a + 1:b + 1, :], iT_d[0, :, a + 1:b + 1, :])
                nc.scalar.dma_start(wf1[:, a:b, :], w_fi1_d[:, a:b, :])
            nc.sync.dma_start(wc2[:], w_ct1_d[:])
            nc.sync.dma_start(wc1[:], w_ci1_d[:])
            nc.sync.dma_start(x_1[:, 0:9, :], iT_d[1, :, 0:9, :])
            nc.sync.dma_start(x_1[:, 9:XALL, :], iT_d[1, :, 9:XALL, :])
            nc.scalar.dma_start(wV[:, 0:4, :], w_V_d[:, 0:4, :])
            nc.scalar.dma_start(wV[:, 4:8, :], w_V_d[:, 4:8, :])
            nc.scalar.dma_start(wT[:, 0:4, :], w_T_d[:, 0:4, :])
            nc.scalar.dma_start(wT[:, 4:8, :], w_T_d[:, 4:8, :])

            def act(ps, htag, n, m, bcol, func):
                h = hpool.tile([128, NT], f16, tag=htag, name=f"{htag}_{n}_{m}")
                nc.scalar.activation(h[:], ps[:], func, bias=bt[:, bcol + m:bcol + m + 1])
                return h

            def layer_k_outer(wt, xs, htag, n, bcol, K, absorb=()):
                """All 4 output blocks accumulate in parallel, k outer: 4
                matmuls per input chunk k (rate-matches chunked DMA arrival).

                absorb: after these k, emit 2 throwaway warm matmuls — they
                soak up jittery DMA-arrival stalls (8 cores contend for HBM
                during startup) so the PE clock never drops to mid p-state.
                """
                aps = None
                if absorb:
                    # one ring slot, reused for every absorber pair (a fresh
                    # slot per pair would wrap onto this layer's own open
                    # accumulators and deadlock the in-order PE queue)
                    aps = pspool.tile([128, NT], f32, tag="ps", name=f"abs_{htag}_{n}")
                ps = [pspool.tile([128, NT], f32, tag="ps", name=f"ps_{htag}_{n}_{m}")
                      for m in range(4)]
                for k in range(K):
                    for m in range(4):
                        nc.tensor.matmul(ps[m][:], wt[:, k, m * 128:(m + 1) * 128],
                                         xs[k], start=(k == 0), stop=(k == K - 1))
                    if k in absorb:
                        for _ in range(2):
                            nc.tensor.matmul(aps[:], warm[:, 0:128], warm[:],
                                             start=True, stop=True)
                return [act(ps[m], htag, n, m, bcol, Relu) for m in range(4)]

            def layer_m_outer(wt, xs, htag, n, bcol, K):
                """m outer: each PSUM bank closes after its k loop and drains
                on the scalar engine while the PE works on the next block."""
                outs = []
                for m in range(4):
                    ps = pspool.tile([128, NT], f32, tag="ps", name=f"ps_{htag}_{n}_{m}")
                    for k in range(K):
                        nc.tensor.matmul(ps[:], wt[:, k, m * 128:(m + 1) * 128],
                                         xs[k], start=(k == 0), stop=(k == K - 1))
                    outs.append(act(ps, htag, n, m, bcol, Relu))
                return outs

            def cat_layer(wt, xs_a, xs_b, n, bcol, oname, orow0, final=False):
                """out[m] = sum_k w[k].T@xs_a[k] + w[4+k].T@xs_b[k] + bias; f32 store.

                The bias-add alternates between the scalar and (otherwise idle)
                vector engines so the final tile's four output blocks drain in
                parallel; stores issue from the sync queue, idle by then. The
                very last store (final, m=3) goes in two column halves on two
                queues so the closing transfer is half as long.
                """
                for m in range(4):
                    ps = pspool.tile([128, NT], f32, tag="ps", name=f"ps_{oname}_{n}_{m}")
                    for k in range(4):
                        nc.tensor.matmul(ps[:], wt[:, k, m * 128:(m + 1) * 128],
                                         xs_a[k], start=(k == 0), stop=False)
                    for k in range(4):
                        nc.tensor.matmul(ps[:], wt[:, 4 + k, m * 128:(m + 1) * 128],
                                         xs_b[k], start=False, stop=(k == 3))
                    o = opool.tile([128, NT], f32, tag="o", name=f"o{oname}_{n}_{m}")
                    bias_ap = bt[:, bcol + m:bcol + m + 1]
                    # the act gating the kernel's last store goes on scalar
                    # (slightly faster than the DVE add)
                    if (m % 2 == 0) or (final and m == 3):
                        nc.scalar.activation(o[:], ps[:], Ident, bias=bias_ap)
                    else:
                        nc.vector.tensor_scalar_add(o[:], ps[:], bias_ap)
                    rows = out_d[orow0 + 128 * m:orow0 + 128 * (m + 1),
                                 n * NT:(n + 1) * NT]
                    if final and m == 3:
                        nc.sync.dma_start(rows[:, 0:NT // 2], o[:, 0:NT // 2])
                        nc.scalar.dma_start(rows[:, NT // 2:NT], o[:, NT // 2:NT])
                    else:
                        nc.sync.dma_start(rows, o[:])

            def vt_phase(n, v1, i_, v2, t_, final=False):
                # ---- V = [v1, i_] @ WcatV.T + bcatV ----
                cat_layer(wV, [h[:] for h in v1], [h[:] for h in i_], n, 16, "V", 0)
                # ---- T = [v2, t_] @ WcatT.T + bcatT ----
                cat_layer(wT, [h[:] for h in v2], [h[:] for h in t_], n, 20, "T", HID,
                          final=final)

            xtiles = [x_cur, x_1]
            prev = None
            for n in range(ntiles):
                x_n = xtiles[n]
                xs_i = [x_n[:, k + 1, :] for k in range(KI)]

                # ---- t_ = relu(t @ ft1.T + b): gates on only 256KB of input ----
                t_ = layer_m_outer(wt1, [x_n[:, 0, :]], "t_", n, 4, 1)
                # ---- i_ = relu(i @ fi1.T + b) ----
                i_ = layer_k_outer(wf1, xs_i, "i_", n, 0, KI,
                                   absorb=(1, 2, 3, 4, 5) if n == 0 else ())

                # prefetch tile n+2 (tile 1's prefetch rode the preamble)
                if n + 2 < ntiles:
                    x_nxt = xpool.tile([128, XALL, NT], f16, tag="x", name=f"x_{n + 2}")
                    nc.sync.dma_start(x_nxt[:, 0:9, :], iT_d[n + 2, :, 0:9, :])
                    nc.sync.dma_start(x_nxt[:, 9:XALL, :], iT_d[n + 2, :, 9:XALL, :])
                    xtiles.append(x_nxt)

                # ---- v2 = relu(t_ @ ct1.T + b): fills the i_ activation latency ----
                v2 = layer_m_outer(wc2, [h[:] for h in t_], "v2", n, 12, 4)
                # ---- v1 = relu(i_ @ ci1.T + b) ----
                v1 = layer_m_outer(wc1, [h[:] for h in i_], "v1", n, 8, 4)

                # ---- V/T of the PREVIOUS tile (one-tile software pipeline:
                # moves the wV/wT load deadlines out of the startup crunch) ----
                if prev is not None:
                    vt_phase(n - 1, *prev)
                prev = (v1, i_, v2, t_)

            vt_phase(ntiles - 1, *prev, final=True)

    nc.compile()
    return nc


def _host_pack(inp: dict):
    f8 = np.float64
    fi1_w, fi1_b = inp["fi1_w"], inp["fi1_b"]
    ft1_w, ft1_b = inp["ft1_w"], inp["ft1_b"]
    ci1_w, ci1_b = inp["ci1_w"], inp["ci1_b"]
    ct1_w, ct1_b = inp["ct1_w"], inp["ct1_b"]

    def fold(wv, bv, wo, bo, f_w, f_b):
        Wvo = wo.astype(f8) @ wv.astype(f8)
        bvo = wo.astype(f8) @ bv.astype(f8) + bo.astype(f8)
        Wcat = np.concatenate([f_w.astype(f8) @ Wvo, f_w.astype(f8)], axis=1)
        bcat = f_w.astype(f8) @ bvo + f_b.astype(f8)
        return Wcat.astype(np.float32), bcat.astype(np.float32)

    WcatV, bcatV = fold(inp["aV_wv"], inp["aV_bv"], inp["aV_wo"], inp["aV_bo"],
                        inp["fi2_w"], inp["fi2_b"])
    WcatT, bcatT = fold(inp["aT_wv"], inp["aT_bv"], inp["aT_wo"], inp["aT_bo"],
                        inp["ft2_w"], inp["ft2_b"])

    weights = {
        "w_fi1": _pack_blocks(np.ascontiguousarray(fi1_w.T), 16, 4),
        "w_ft1": _pack_blocks(np.ascontiguousarray(ft1_w.T), 1, 4),
        "w_ci1": _pack_blocks(np.ascontiguousarray(ci1_w.T), 4, 4),
        "w_ct1": _pack_blocks(np.ascontiguousarray(ct1_w.T), 4, 4),
        "w_V": _pack_blocks(np.ascontiguousarray(WcatV.T), 8, 4),
        "w_T": _pack_blocks(np.ascontiguousarray(WcatT.T), 8, 4),
    }
    cols = []
    for b in (fi1_b, ft1_b, ci1_b, ct1_b, bcatV, bcatT):
        for m in range(4):
            cols.append(b[128 * m:128 * (m + 1)])
    weights["bias"] = np.ascontiguousarray(np.stack(cols, axis=1), dtype=np.float32)
    return weights


def make_in_maps(inputs: dict):
    """Full inputs -> per-core input dicts (shard batch, replicate weights)."""
    inputs = {k: np.asarray(v) for k, v in inputs.items()}
    i = np.asarray(inputs["i"], dtype=np.float32)
    t = np.asarray(inputs["t"], dtype=np.float32)
    weights = _host_pack(inputs)
    i16 = i.astype(np.float16)
    t16 = t.astype(np.float16)
    ntiles = BS // NT
    in_maps = []
    for c in range(NCORES):
        sl = slice(c * BS, (c + 1) * BS)
        m = dict(weights)
        # [ntiles, 128, XALL, NT]: batch-tile major; chunk 0 holds
        # t[n*NT+j, p], chunk 1+k holds i[n*NT+j, 128k+p].
        xi = i16[sl].T.reshape(KI, 128, ntiles, NT)   # [k, p, n, j]
        xt = t16[sl].T.reshape(TAB, ntiles, NT)       # [p, n, j]
        full = np.empty((ntiles, 128, XALL, NT), dtype=np.float16)
        full[:, :, 0, :] = xt.transpose(1, 0, 2)
        full[:, :, 1:, :] = xi.transpose(2, 1, 0, 3)
        m["iT"] = full
        in_maps.append(m)
    return in_maps


def kernel(**inputs) -> np.ndarray:
    from concourse import bass_utils

    if "nc" not in _CACHE:
        _CACHE["nc"] = _build_nc(BS)
    nc = _CACHE["nc"]

    in_maps = make_in_maps(inputs)
    res = bass_utils.run_bass_kernel_spmd(nc, in_maps, core_ids=list(range(NCORES)))

    out = np.empty((B, 2 * HID), dtype=np.float32)
    for c in range(NCORES):
        out[c * BS:(c + 1) * BS] = res.results[c]["outT"].T
    return out



# revision 8
# speedup vs baseline: 1.0067x; 1.0067x over previous
"""Trainium2 Bass kernel for nn_CMFA (dense_transformer, seq_len=1 cross-attention).

Math notes (exact simplifications vs the reference):
  - softmax over a single key is exactly 1.0, so the attention output is
    exactly the v-projection: mha(q,k,v) = (v @ Wv.T + bv) @ Wo.T + bo.
    The q/k projections never influence the output.
  - Wv -> Wo -> fi2 is a linear chain (no nonlinearity), so it is folded on
    the host:  V = [v1, i_] @ Wcat.T + bcat  with
      Wcat = [fi2 @ (Wo @ Wv), fi2],  bcat = fi2 @ (Wo @ bv + bo) + fi2_b
    (the i_ column block carries the residual through fi2).

Precision: matmul operands (inputs, weights, intermediate activations) are
fp16; PSUM accumulation, biases and the final output are fp32. fp16 keeps
the PE at 1 cycle/row (same as f32r at 512-wide moving dim) but halves the
stationary-weight load so it hides under the previous matmul's streaming
window (f32r pays ~25ns per matmul for it), and halves HBM traffic.
Measured end-to-end error vs the f32 reference is ~5e-4 (gate is 2e-2).

Device layout: activations are feature-major ("transposed", [feat, batch]) so
every matmul contracts over the partition dim and every DMA is contiguous.
The host pre-transposes the batch shards of i/t and transposes the output
back. Pure data parallel across 8 cores; weights replicated.

Schedule notes (all from trace measurements):
  - All loads ride the sync queue in program order: each DMA issue costs
    ~610ns of sequencer time, so tile-0 x / fi1-weight chunks are grouped
    (1/2/3/4/6) to stay ahead of the PE's 853ns-per-chunk consumption, and
    prefetches are issued behind the preamble so they cannot steal DMA
    bandwidth from startup-critical transfers.
  - Output stores issue from the scalar queue (hardware DGE, and the act
    that produces the tile runs there, so no cross-engine hop); the gpsimd
    queue's software DGE costs ~1us per store and serializes the tail.
  - The PE drops to half clock for ~3us after any idle gap, so a dozen
    warm-up matmuls on a memset tile bridge the DMA startup window and the
    real stream enters at full clock.
  - t-branch first (ft1 gates on only 192KB of input), then fi1 (k-outer:
    4 matmuls per arriving x chunk), then ct1/ci1 (hides the i_ activation
    latency), then the two folded output layers.
"""

import numpy as np

B, IMG, TAB, HID = 32768, 2048, 128, 512
NCORES = 8
BS = B // NCORES  # rows per core
NT = 512          # batch-tile (matmul moving/free dim)
KI = IMG // 128   # 16 contraction chunks for fi1
XALL = KI + 1     # + the t chunk, packed as chunk 0 of the same tile
NWARM = 3         # PE p-state warm-up matmuls (on the wt1 tile, earliest load)

_CACHE = {}


def _pack_blocks(WT: np.ndarray, K: int, M: int) -> np.ndarray:
    """[K*128, M*128] -> [128, K, M*128] with [p, k, m*128+j] = WT[k*128+p, m*128+j]."""
    out = WT.reshape(K, 128, M * 128).transpose(1, 0, 2)
    return np.ascontiguousarray(out, dtype=np.float16)


def _build_nc(bs: int):
    import concourse.bass as bass
    import concourse.tile as tile
    from concourse import bacc, mybir

    f32 = mybir.dt.float32
    f16 = mybir.dt.float16
    Relu = mybir.ActivationFunctionType.Relu
    Ident = mybir.ActivationFunctionType.Identity
    ntiles = bs // NT

    nc = bacc.Bacc("TRN2", target_bir_lowering=False, debug=False)

    # tile-major input layout: per batch-tile, all 17 chunks contiguous per
    # partition (16KB lines -> large DMA descriptors, one prefetch per tile)
    iT_d = nc.dram_tensor("iT", [bs // NT, 128, XALL, NT], f16,
                          kind="ExternalInput").ap()
    w_fi1_d = nc.dram_tensor("w_fi1", [128, KI, 512], f16, kind="ExternalInput").ap()
    w_ft1_d = nc.dram_tensor("w_ft1", [128, 1, 512], f16, kind="ExternalInput").ap()
    w_ci1_d = nc.dram_tensor("w_ci1", [128, 4, 512], f16, kind="ExternalInput").ap()
    w_ct1_d = nc.dram_tensor("w_ct1", [128, 4, 512], f16, kind="ExternalInput").ap()
    w_V_d = nc.dram_tensor("w_V", [128, 8, 512], f16, kind="ExternalInput").ap()
    w_T_d = nc.dram_tensor("w_T", [128, 8, 512], f16, kind="ExternalInput").ap()
    bias_d = nc.dram_tensor("bias", [128, 24], f32, kind="ExternalInput").ap()
    # f16 output: halves store traffic and the closing transfer; adds only
    # ~1e-4 to the (2e-2-gated) rel err -- host upcasts to f32.
    out_d = nc.dram_tensor("outT", [2 * HID, bs], f16, kind="ExternalOutput").ap()

    # fi1 chunk groups for tile 0 (i-chunk indices): sized so grouped DMA
    # issues stay ahead of the PE eating 4 matmuls (853ns) per chunk, finer
    # at the back where cumulative transfer time approaches the consumption
    # schedule. x tile chunk index = i-chunk + 1 (chunk 0 is the t chunk).
    WGRP = [(0, 1), (1, 3), (3, 6), (6, 9), (9, 12), (12, 14), (14, 16)]

    with tile.TileContext(nc) as tc:
        with (
            tc.tile_pool(name="w", bufs=1) as wpool,
            tc.tile_pool(name="x", bufs=3) as xpool,
            tc.tile_pool(name="h", bufs=8) as hpool,
            tc.tile_pool(name="o", bufs=8) as opool,
            tc.tile_pool(name="ps", bufs=8, space="PSUM") as pspool,
        ):
            wf1 = wpool.tile([128, KI, 512], f16, name="w_fi1_t")
            wt1 = wpool.tile([128, 1, 512], f16, name="w_ft1_t")
            wc1 = wpool.tile([128, 4, 512], f16, name="w_ci1_t")
            wc2 = wpool.tile([128, 4, 512], f16, name="w_ct1_t")
            wV = wpool.tile([128, 8, 512], f16, name="w_V_t")
            wT = wpool.tile([128, 8, 512], f16, name="w_T_t")
            bt = wpool.tile([128, 24], f32, name="bias_t")

            # ---- preamble loads, earliest-deadline order, split across the
            # sync and scalar queues so issue cadence (~610ns each, in-order
            # per queue) never falls behind the PE's chunk consumption.
            # wt1 rides first so the warm-up matmuls (which read it) can
            # start ~1.6us in, with no DVE-memset dependency chain.
            # V/T of tile n are deferred into tile n+1, so wV/wT are not
            # needed until deep into tile 1, after the startup HBM crunch.
            x_cur = xpool.tile([128, XALL, NT], f16, tag="x", name="x_0")
            x_1 = xpool.tile([128, XALL, NT], f16, tag="x", name="x_1")
            nc.sync.dma_start(wt1[:], w_ft1_d[:])
            nc.sync.dma_start(x_cur[:, 0, :], iT_d[0, :, 0, :])
            nc.scalar.dma_start(bt[:], bias_d[:])
            for a, b in WGRP:
                nc.sync.dma_start(x_cur[:, a + 1:b + 1, :], iT_d[0, :, a + 1:b + 1, :])
                nc.scalar.dma_start(wf1[:, a:b, :], w_fi1_d[:, a:b, :])

            # ---- PE warm-up on wt1: starts the HAM busy window early and
            # covers the remaining x/bias DMA latency at rising clock ----
            wps = pspool.tile([128, NT], f32, tag="ps", name="warm_ps")
            for _ in range(NWARM):
                nc.tensor.matmul(wps[:], wt1[:, 0, 0:128], wt1[:, 0, :],
                                 start=True, stop=True)
            nc.sync.dma_start(wc2[:], w_ct1_d[:])
            nc.sync.dma_start(wc1[:], w_ci1_d[:])
            nc.sync.dma_start(x_1[:, 0:9, :], iT_d[1, :, 0:9, :])
            nc.sync.dma_start(x_1[:, 9:XALL, :], iT_d[1, :, 9:XALL, :])
            nc.scalar.dma_start(wV[:, 0:4, :], w_V_d[:, 0:4, :])
            nc.scalar.dma_start(wV[:, 4:8, :], w_V_d[:, 4:8, :])
            nc.scalar.dma_start(wT[:, 0:4, :], w_T_d[:, 0:4, :])
            nc.scalar.dma_start(wT[:, 4:8, :], w_T_d[:, 4:8, :])

            def act(ps, htag, n, m, bcol, func):
                h = hpool.tile([128, NT], f16, tag=htag, name=f"{htag}_{n}_{m}")
                nc.scalar.activation(h[:], ps[:], func, bias=bt[:, bcol + m:bcol + m + 1])
                return h

            def layer_k_outer(wt, xs, htag, n, bcol, K):
                """All 4 output blocks accumulate in parallel, k outer: 4
                matmuls per input chunk k (rate-matches chunked DMA arrival)."""
                ps = [pspool.tile([128, NT], f32, tag="ps", name=f"ps_{htag}_{n}_{m}")
                      for m in range(4)]
                for k in range(K):
                    for m in range(4):
                        nc.tensor.matmul(ps[m][:], wt[:, k, m * 128:(m + 1) * 128],
                                         xs[k], start=(k == 0), stop=(k == K - 1))
                return [act(ps[m], htag, n, m, bcol, Relu) for m in range(4)]

            def layer_m_outer(wt, xs, htag, n, bcol, K):
                """m outer: each PSUM bank closes after its k loop and drains
                on the scalar engine while the PE works on the next block."""
                outs = []
                for m in range(4):
                    ps = pspool.tile([128, NT], f32, tag="ps", name=f"ps_{htag}_{n}_{m}")
                    for k in range(K):
                        nc.tensor.matmul(ps[:], wt[:, k, m * 128:(m + 1) * 128],
                                         xs[k], start=(k == 0), stop=(k == K - 1))
                    outs.append(act(ps, htag, n, m, bcol, Relu))
                return outs

            def cat_layer(wt, xs_a, xs_b, n, bcol, oname, orow0, final=False):
                """out[m] = sum_k w[k].T@xs_a[k] + w[4+k].T@xs_b[k] + bias; f32 store.

                The bias-add alternates between the scalar and (otherwise idle)
                vector engines so the final tile's four output blocks drain in
                parallel; stores issue from the sync queue, idle by then. The
                very last store (final, m=3) goes in two column halves on two
                queues so the closing transfer is half as long.
                """
                for m in range(4):
                    ps = pspool.tile([128, NT], f32, tag="ps", name=f"ps_{oname}_{n}_{m}")
                    for k in range(4):
                        nc.tensor.matmul(ps[:], wt[:, k, m * 128:(m + 1) * 128],
                                         xs_a[k], start=(k == 0), stop=False)
                    for k in range(4):
                        nc.tensor.matmul(ps[:], wt[:, 4 + k, m * 128:(m + 1) * 128],
                                         xs_b[k], start=False, stop=(k == 3))
                    o = opool.tile([128, NT], f16, tag="o", name=f"o{oname}_{n}_{m}")
                    bias_ap = bt[:, bcol + m:bcol + m + 1]
                    # the act gating the kernel's last store goes on scalar
                    # (slightly faster than the DVE add)
                    if (m % 2 == 0) or (final and m == 3):
                        nc.scalar.activation(o[:], ps[:], Ident, bias=bias_ap)
                    else:
                        nc.vector.tensor_scalar_add(o[:], ps[:], bias_ap)
                    rows = out_d[orow0 + 128 * m:orow0 + 128 * (m + 1),
                                 n * NT:(n + 1) * NT]
                    if final and m == 3:
                        nc.sync.dma_start(rows[:, 0:NT // 2], o[:, 0:NT // 2])
                        nc.scalar.dma_start(rows[:, NT // 2:NT], o[:, NT // 2:NT])
                    else:
                        nc.sync.dma_start(rows, o[:])

            def vt_phase(n, v1, i_, v2, t_, final=False):
                # ---- V = [v1, i_] @ WcatV.T + bcatV ----
                cat_layer(wV, [h[:] for h in v1], [h[:] for h in i_], n, 16, "V", 0)
                # ---- T = [v2, t_] @ WcatT.T + bcatT ----
                cat_layer(wT, [h[:] for h in v2], [h[:] for h in t_], n, 20, "T", HID,
                          final=final)

            xtiles = [x_cur, x_1]
            prev = None
            for n in range(ntiles):
                x_n = xtiles[n]
                xs_i = [x_n[:, k + 1, :] for k in range(KI)]

                # ---- t_ = relu(t @ ft1.T + b): gates on only 256KB of input ----
                t_ = layer_m_outer(wt1, [x_n[:, 0, :]], "t_", n, 4, 1)
                # ---- i_ = relu(i @ fi1.T + b) ----
                i_ = layer_k_outer(wf1, xs_i, "i_", n, 0, KI)

                # prefetch tile n+2 (tile 1's prefetch rode the preamble)
                if n + 2 < ntiles:
                    x_nxt = xpool.tile([128, XALL, NT], f16, tag="x", name=f"x_{n + 2}")
                    nc.sync.dma_start(x_nxt[:, 0:9, :], iT_d[n + 2, :, 0:9, :])
                    nc.sync.dma_start(x_nxt[:, 9:XALL, :], iT_d[n + 2, :, 9:XALL, :])
                    xtiles.append(x_nxt)

                # ---- v2 = relu(t_ @ ct1.T + b): fills the i_ activation latency ----
                v2 = layer_m_outer(wc2, [h[:] for h in t_], "v2", n, 12, 4)
                # ---- v1 = relu(i_ @ ci1.T + b) ----
                v1 = layer_m_outer(wc1, [h[:] for h in i_], "v1", n, 8, 4)

                # ---- V/T of the PREVIOUS tile (one-tile software pipeline:
                # moves the wV/wT load deadlines out of the startup crunch) ----
                if prev is not None:
                    vt_phase(n - 1, *prev)
                prev = (v1, i_, v2, t_)

            vt_phase(ntiles - 1, *prev, final=True)

    nc.compile()
    return nc


def _host_pack(inp: dict):
    f8 = np.float64
    fi1_w, fi1_b = inp["fi1_w"], inp["fi1_b"]
    ft1_w, ft1_b = inp["ft1_w"], inp["ft1_b"]
    ci1_w, ci1_b = inp["ci1_w"], inp["ci1_b"]
    ct1_w, ct1_b = inp["ct1_w"], inp["ct1_b"]

    def fold(wv, bv, wo, bo, f_w, f_b):
        Wvo = wo.astype(f8) @ wv.astype(f8)
        bvo = wo.astype(f8) @ bv.astype(f8) + bo.astype(f8)
        Wcat = np.concatenate([f_w.astype(f8) @ Wvo, f_w.astype(f8)], axis=1)
        bcat = f_w.astype(f8) @ bvo + f_b.astype(f8)
        return Wcat.astype(np.float32), bcat.astype(np.float32)

    WcatV, bcatV = fold(inp["aV_wv"], inp["aV_bv"], inp["aV_wo"], inp["aV_bo"],
                        inp["fi2_w"], inp["fi2_b"])
    WcatT, bcatT = fold(inp["aT_wv"], inp["aT_bv"], inp["aT_wo"], inp["aT_bo"],
                        inp["ft2_w"], inp["ft2_b"])

    weights = {
        "w_fi1": _pack_blocks(np.ascontiguousarray(fi1_w.T), 16, 4),
        "w_ft1": _pack_blocks(np.ascontiguousarray(ft1_w.T), 1, 4),
        "w_ci1": _pack_blocks(np.ascontiguousarray(ci1_w.T), 4, 4),
        "w_ct1": _pack_blocks(np.ascontiguousarray(ct1_w.T), 4, 4),
        "w_V": _pack_blocks(np.ascontiguousarray(WcatV.T), 8, 4),
        "w_T": _pack_blocks(np.ascontiguousarray(WcatT.T), 8, 4),
    }
    cols = []
    for b in (fi1_b, ft1_b, ci1_b, ct1_b, bcatV, bcatT):
        for m in range(4):
            cols.append(b[128 * m:128 * (m + 1)])
    weights["bias"] = np.ascontiguousarray(np.stack(cols, axis=1), dtype=np.float32)
    return weights


def make_in_maps(inputs: dict):
    """Full inputs -> per-core input dicts (shard batch, replicate weights)."""
    inputs = {k: np.asarray(v) for k, v in inputs.items()}
    i = np.asarray(inputs["i"], dtype=np.float32)
    t = np.asarray(inputs["t"], dtype=np.float32)
    weights = _host_pack(inputs)
    i16 = i.astype(np.float16)
    t16 = t.astype(np.float16)
    ntiles = BS // NT
    in_maps = []
    for c in range(NCORES):
        sl = slice(c * BS, (c + 1) * BS)
        m = dict(weights)
        # [ntiles, 128, XALL, NT]: batch-tile major; chunk 0 holds
        # t[n*NT+j, p], chunk 1+k holds i[n*NT+j, 128k+p].
        xi = i16[sl].T.reshape(KI, 128, ntiles, NT)   # [k, p, n, j]
        xt = t16[sl].T.reshape(TAB, ntiles, NT)       # [p, n, j]
        full = np.empty((ntiles, 128, XALL, NT), dtype=np.float16)
        full[:, :, 0, :] = xt.transpose(1, 0, 2)
        full[:, :, 1:, :] = xi.transpose(2, 1, 0, 3)
        m["iT"] = full
        in_maps.append(m)
    return in_maps


def kernel(**inputs) -> np.ndarray:
    from concourse import bass_utils

    if "nc" not in _CACHE:
        _CACHE["nc"] = _build_nc(BS)
    nc = _CACHE["nc"]

    in_maps = make_in_maps(inputs)
    res = bass_utils.run_bass_kernel_spmd(nc, in_maps, core_ids=list(range(NCORES)))

    out = np.empty((B, 2 * HID), dtype=np.float32)
    for c in range(NCORES):
        out[c * BS:(c + 1) * BS] = res.results[c]["outT"].T.astype(np.float32)
    return out



# revision 13
# speedup vs baseline: 1.0131x; 1.0064x over previous
"""Trainium2 Bass kernel for nn_CMFA (dense_transformer, seq_len=1 cross-attention).

Math notes (exact simplifications vs the reference):
  - softmax over a single key is exactly 1.0, so the attention output is
    exactly the v-projection: mha(q,k,v) = (v @ Wv.T + bv) @ Wo.T + bo.
    The q/k projections never influence the output.
  - Wv -> Wo -> fi2 is a linear chain (no nonlinearity), so it is folded on
    the host:  V = [v1, i_] @ Wcat.T + bcat  with
      Wcat = [fi2 @ (Wo @ Wv), fi2],  bcat = fi2 @ (Wo @ bv + bo) + fi2_b
    (the i_ column block carries the residual through fi2).

Precision: matmul operands (inputs, weights, intermediate activations) are
fp16; PSUM accumulation, biases and the final output are fp32. fp16 keeps
the PE at 1 cycle/row (same as f32r at 512-wide moving dim) but halves the
stationary-weight load so it hides under the previous matmul's streaming
window (f32r pays ~25ns per matmul for it), and halves HBM traffic.
Measured end-to-end error vs the f32 reference is ~5e-4 (gate is 2e-2).

Device layout: activations are feature-major ("transposed", [feat, batch]) so
every matmul contracts over the partition dim and every DMA is contiguous.
The host pre-transposes the batch shards of i/t and transposes the output
back. Pure data parallel across 8 cores; weights replicated.

Schedule notes (all from trace measurements):
  - All loads ride the sync queue in program order: each DMA issue costs
    ~610ns of sequencer time, so tile-0 x / fi1-weight chunks are grouped
    (1/2/3/4/6) to stay ahead of the PE's 853ns-per-chunk consumption, and
    prefetches are issued behind the preamble so they cannot steal DMA
    bandwidth from startup-critical transfers.
  - Output stores issue from the scalar queue (hardware DGE, and the act
    that produces the tile runs there, so no cross-engine hop); the gpsimd
    queue's software DGE costs ~1us per store and serializes the tail.
  - The PE drops to half clock for ~3us after any idle gap, so a dozen
    warm-up matmuls on a memset tile bridge the DMA startup window and the
    real stream enters at full clock.
  - t-branch first (ft1 gates on only 192KB of input), then fi1 (k-outer:
    4 matmuls per arriving x chunk), then ct1/ci1 (hides the i_ activation
    latency), then the two folded output layers.
"""

import numpy as np

B, IMG, TAB, HID = 32768, 2048, 128, 512
NCORES = 8
BS = B // NCORES  # rows per core
NT = 512          # batch-tile (matmul moving/free dim)
KI = IMG // 128   # 16 contraction chunks for fi1
XALL = KI + 1     # + the t chunk, packed as chunk 0 of the same tile
NWARM = 2         # PE p-state warm-up matmuls (on the wt1 tile, earliest load)

_CACHE = {}


def _pack_blocks(WT: np.ndarray, K: int, M: int) -> np.ndarray:
    """[K*128, M*128] -> [128, K, M*128] with [p, k, m*128+j] = WT[k*128+p, m*128+j]."""
    out = WT.reshape(K, 128, M * 128).transpose(1, 0, 2)
    return np.ascontiguousarray(out, dtype=np.float16)


def _build_nc(bs: int):
    import concourse.bass as bass
    import concourse.tile as tile
    from concourse import bacc, mybir

    f32 = mybir.dt.float32
    f16 = mybir.dt.float16
    Relu = mybir.ActivationFunctionType.Relu
    Ident = mybir.ActivationFunctionType.Identity
    ntiles = bs // NT

    nc = bacc.Bacc("TRN2", target_bir_lowering=False, debug=False)

    # tile-major input layout: per batch-tile, all 17 chunks contiguous per
    # partition (16KB lines -> large DMA descriptors, one prefetch per tile)
    iT_d = nc.dram_tensor("iT", [bs // NT, 128, XALL, NT], f16,
                          kind="ExternalInput").ap()
    w_fi1_d = nc.dram_tensor("w_fi1", [128, KI, 512], f16, kind="ExternalInput").ap()
    w_ft1_d = nc.dram_tensor("w_ft1", [128, 1, 512], f16, kind="ExternalInput").ap()
    w_ci1_d = nc.dram_tensor("w_ci1", [128, 4, 512], f16, kind="ExternalInput").ap()
    w_ct1_d = nc.dram_tensor("w_ct1", [128, 4, 512], f16, kind="ExternalInput").ap()
    w_V_d = nc.dram_tensor("w_V", [128, 8, 512], f16, kind="ExternalInput").ap()
    w_T_d = nc.dram_tensor("w_T", [128, 8, 512], f16, kind="ExternalInput").ap()
    bias_d = nc.dram_tensor("bias", [128, 24], f32, kind="ExternalInput").ap()
    # f16 output: halves store traffic and the closing transfer; adds only
    # ~1e-4 to the (2e-2-gated) rel err -- host upcasts to f32.
    out_d = nc.dram_tensor("outT", [2 * HID, bs], f16, kind="ExternalOutput").ap()

    # fi1 chunk groups for tile 0 (i-chunk indices): sized so grouped DMA
    # issues stay ahead of the PE eating 4 matmuls (853ns) per chunk, finer
    # at the back where cumulative transfer time approaches the consumption
    # schedule. x tile chunk index = i-chunk + 1 (chunk 0 is the t chunk).
    WGRP = [(0, 1), (1, 3), (3, 6), (6, 9), (9, 12), (12, 14), (14, 16)]

    with tile.TileContext(nc) as tc:
        with (
            tc.tile_pool(name="w", bufs=1) as wpool,
            tc.tile_pool(name="x", bufs=2) as xpool,
            tc.tile_pool(name="h", bufs=8) as hpool,
            tc.tile_pool(name="o", bufs=8) as opool,
            tc.tile_pool(name="ps", bufs=8, space="PSUM") as pspool,
        ):
            wf1 = wpool.tile([128, KI, 512], f16, name="w_fi1_t")
            wt1 = wpool.tile([128, 1, 512], f16, name="w_ft1_t")
            wc1 = wpool.tile([128, 4, 512], f16, name="w_ci1_t")
            wc2 = wpool.tile([128, 4, 512], f16, name="w_ct1_t")
            wV = wpool.tile([128, 8, 512], f16, name="w_V_t")
            wT = wpool.tile([128, 8, 512], f16, name="w_T_t")
            bt = wpool.tile([128, 24], f32, name="bias_t")

            # ---- preamble loads: ONLY what tile 0's t_/i_ layers need.
            # 8 cores flood the shared HBM at startup, so every byte issued
            # here delays the startup-critical chunks on every core.  All
            # later-needed tensors (wc, wV/wT, x tile 1) are issued from the
            # scalar queue BEHIND act instructions inside the n=0 body: the
            # scalar queue is FIFO and acts wait on PE sems, so those
            # transfers only start once tile 0 is well underway.
            x_cur = xpool.tile([128, XALL, NT], f16, tag="x", name="x_0")
            x_1 = xpool.tile([128, XALL, NT], f16, tag="x", name="x_1")
            nc.sync.dma_start(wt1[:], w_ft1_d[:])
            nc.sync.dma_start(x_cur[:, 0, :], iT_d[0, :, 0, :])
            nc.scalar.dma_start(bt[:], bias_d[:])
            for a, b in WGRP:
                nc.sync.dma_start(x_cur[:, a + 1:b + 1, :], iT_d[0, :, a + 1:b + 1, :])
                nc.scalar.dma_start(wf1[:, a:b, :], w_fi1_d[:, a:b, :])

            # ---- PE warm-up on wt1: starts the HAM busy window early and
            # covers the remaining x/bias DMA latency at rising clock ----
            wps = pspool.tile([128, NT], f32, tag="ps", name="warm_ps")
            for _ in range(NWARM):
                nc.tensor.matmul(wps[:], wt1[:, 0, 0:128], wt1[:, 0, :],
                                 start=True, stop=True)

            def act(ps, htag, n, m, bcol, func):
                h = hpool.tile([128, NT], f16, tag=htag, name=f"{htag}_{n}_{m}")
                nc.scalar.activation(h[:], ps[:], func, bias=bt[:, bcol + m:bcol + m + 1])
                return h

            def layer_k_outer(wt, xs, htag, n, bcol, K):
                """All 4 output blocks accumulate in parallel, k outer: 4
                matmuls per input chunk k (rate-matches chunked DMA arrival)."""
                ps = [pspool.tile([128, NT], f32, tag="ps", name=f"ps_{htag}_{n}_{m}")
                      for m in range(4)]
                for k in range(K):
                    for m in range(4):
                        nc.tensor.matmul(ps[m][:], wt[:, k, m * 128:(m + 1) * 128],
                                         xs[k], start=(k == 0), stop=(k == K - 1))
                return [act(ps[m], htag, n, m, bcol, Relu) for m in range(4)]

            def layer_m_outer(wt, xs, htag, n, bcol, K):
                """m outer: each PSUM bank closes after its k loop and drains
                on the scalar engine while the PE works on the next block."""
                outs = []
                for m in range(4):
                    ps = pspool.tile([128, NT], f32, tag="ps", name=f"ps_{htag}_{n}_{m}")
                    for k in range(K):
                        nc.tensor.matmul(ps[:], wt[:, k, m * 128:(m + 1) * 128],
                                         xs[k], start=(k == 0), stop=(k == K - 1))
                    outs.append(act(ps, htag, n, m, bcol, Relu))
                return outs

            def cat_layer(wt, xs_a, xs_b, n, bcol, oname, orow0, final=False):
                """out[m] = sum_k w[k].T@xs_a[k] + w[4+k].T@xs_b[k] + bias; f32 store.

                The bias-add alternates between the scalar and (otherwise idle)
                vector engines so the final tile's four output blocks drain in
                parallel; stores issue from the sync queue, idle by then. The
                very last store (final, m=3) goes in two column halves on two
                queues so the closing transfer is half as long.
                """
                for m in range(4):
                    ps = pspool.tile([128, NT], f32, tag="ps", name=f"ps_{oname}_{n}_{m}")
                    for k in range(4):
                        nc.tensor.matmul(ps[:], wt[:, k, m * 128:(m + 1) * 128],
                                         xs_a[k], start=(k == 0), stop=False)
                    for k in range(4):
                        nc.tensor.matmul(ps[:], wt[:, 4 + k, m * 128:(m + 1) * 128],
                                         xs_b[k], start=False, stop=(k == 3))
                    o = opool.tile([128, NT], f16, tag="o", name=f"o{oname}_{n}_{m}")
                    bias_ap = bt[:, bcol + m:bcol + m + 1]
                    rows = out_d[orow0 + 128 * m:orow0 + 128 * (m + 1),
                                 n * NT:(n + 1) * NT]
                    if final and m == 3:
                        # kernel's closing chain: bias-add the two column
                        # halves on scalar+vector in parallel, store each from
                        # its own HWDGE ring as soon as its half is ready
                        H = NT // 2
                        nc.scalar.activation(o[:, 0:H], ps[:, 0:H], Ident,
                                             bias=bias_ap)
                        nc.vector.tensor_scalar_add(o[:, H:NT], ps[:, H:NT],
                                                    bias_ap)
                        nc.sync.dma_start(rows[:, 0:H], o[:, 0:H])
                        nc.scalar.dma_start(rows[:, H:NT], o[:, H:NT])
                    else:
                        if m % 2 == 0:
                            nc.scalar.activation(o[:], ps[:], Ident, bias=bias_ap)
                        else:
                            nc.vector.tensor_scalar_add(o[:], ps[:], bias_ap)
                        nc.sync.dma_start(rows, o[:])

            def vt_phase(n, v1, i_, v2, t_, final=False):
                # ---- V = [v1, i_] @ WcatV.T + bcatV ----
                cat_layer(wV, [h[:] for h in v1], [h[:] for h in i_], n, 16, "V", 0)
                # ---- T = [v2, t_] @ WcatT.T + bcatT ----
                cat_layer(wT, [h[:] for h in v2], [h[:] for h in t_], n, 20, "T", HID,
                          final=final)

            xtiles = [x_cur, x_1]
            prev = None
            for n in range(ntiles):
                x_n = xtiles[n]
                xs_i = [x_n[:, k + 1, :] for k in range(KI)]

                # JIT prefetch of tile n+1 (x pool bufs=2: the issue WARs the
                # tile n-1 slot, so the transfer starts right as tile n does
                # -- a full tile-time (~35us) ahead of need, and never during
                # the 8-core startup HBM crunch)
                if n >= 1 and n + 1 < ntiles:
                    x_nxt = xpool.tile([128, XALL, NT], f16, tag="x", name=f"x_{n + 1}")
                    nc.sync.dma_start(x_nxt[:, 0:9, :], iT_d[n + 1, :, 0:9, :])
                    nc.sync.dma_start(x_nxt[:, 9:XALL, :], iT_d[n + 1, :, 9:XALL, :])
                    xtiles.append(x_nxt)

                # ---- t_ = relu(t @ ft1.T + b): gates on only 256KB of input ----
                t_ = layer_m_outer(wt1, [x_n[:, 0, :]], "t_", n, 4, 1)
                if n == 0:
                    # wc loads ride the scalar queue behind the t_ acts:
                    # transfer starts once tile 0 is underway, lands well
                    # before the v2/v1 layers need them
                    nc.scalar.dma_start(wc2[:], w_ct1_d[:])
                    nc.scalar.dma_start(wc1[:], w_ci1_d[:])
                # ---- i_ = relu(i @ fi1.T + b) ----
                i_ = layer_k_outer(wf1, xs_i, "i_", n, 0, KI)
                if n == 0:
                    # x tile 1 behind the i_ acts (needed from ~tile 1 mid)
                    nc.scalar.dma_start(x_1[:, 0:9, :], iT_d[1, :, 0:9, :])
                    nc.scalar.dma_start(x_1[:, 9:XALL, :], iT_d[1, :, 9:XALL, :])

                # ---- v2 = relu(t_ @ ct1.T + b): fills the i_ activation latency ----
                v2 = layer_m_outer(wc2, [h[:] for h in t_], "v2", n, 12, 4)
                # ---- v1 = relu(i_ @ ci1.T + b) ----
                v1 = layer_m_outer(wc1, [h[:] for h in i_], "v1", n, 8, 4)
                if n == 0:
                    # cat weights behind the v2/v1 acts (needed from the
                    # deferred vt_phase(0), which runs inside tile 1)
                    nc.scalar.dma_start(wV[:, 0:4, :], w_V_d[:, 0:4, :])
                    nc.scalar.dma_start(wV[:, 4:8, :], w_V_d[:, 4:8, :])
                    nc.scalar.dma_start(wT[:, 0:4, :], w_T_d[:, 0:4, :])
                    nc.scalar.dma_start(wT[:, 4:8, :], w_T_d[:, 4:8, :])

                # ---- V/T of the PREVIOUS tile (one-tile software pipeline:
                # moves the wV/wT load deadlines out of the startup crunch) ----
                if prev is not None:
                    vt_phase(n - 1, *prev)
                prev = (v1, i_, v2, t_)

            vt_phase(ntiles - 1, *prev, final=True)

    nc.compile()
    return nc


def _host_pack(inp: dict):
    f8 = np.float64
    fi1_w, fi1_b = inp["fi1_w"], inp["fi1_b"]
    ft1_w, ft1_b = inp["ft1_w"], inp["ft1_b"]
    ci1_w, ci1_b = inp["ci1_w"], inp["ci1_b"]
    ct1_w, ct1_b = inp["ct1_w"], inp["ct1_b"]

    def fold(wv, bv, wo, bo, f_w, f_b):
        Wvo = wo.astype(f8) @ wv.astype(f8)
        bvo = wo.astype(f8) @ bv.astype(f8) + bo.astype(f8)
        Wcat = np.concatenate([f_w.astype(f8) @ Wvo, f_w.astype(f8)], axis=1)
        bcat = f_w.astype(f8) @ bvo + f_b.astype(f8)
        return Wcat.astype(np.float32), bcat.astype(np.float32)

    WcatV, bcatV = fold(inp["aV_wv"], inp["aV_bv"], inp["aV_wo"], inp["aV_bo"],
                        inp["fi2_w"], inp["fi2_b"])
    WcatT, bcatT = fold(inp["aT_wv"], inp["aT_bv"], inp["aT_wo"], inp["aT_bo"],
                        inp["ft2_w"], inp["ft2_b"])

    weights = {
        "w_fi1": _pack_blocks(np.ascontiguousarray(fi1_w.T), 16, 4),
        "w_ft1": _pack_blocks(np.ascontiguousarray(ft1_w.T), 1, 4),
        "w_ci1": _pack_blocks(np.ascontiguousarray(ci1_w.T), 4, 4),
        "w_ct1": _pack_blocks(np.ascontiguousarray(ct1_w.T), 4, 4),
        "w_V": _pack_blocks(np.ascontiguousarray(WcatV.T), 8, 4),
        "w_T": _pack_blocks(np.ascontiguousarray(WcatT.T), 8, 4),
    }
    cols = []
    for b in (fi1_b, ft1_b, ci1_b, ct1_b, bcatV, bcatT):
        for m in range(4):
            cols.append(b[128 * m:128 * (m + 1)])
    weights["bias"] = np.ascontiguousarray(np.stack(cols, axis=1), dtype=np.float32)
    return weights


def make_in_maps(inputs: dict):
    """Full inputs -> per-core input dicts (shard batch, replicate weights)."""
    inputs = {k: np.asarray(v) for k, v in inputs.items()}
    i = np.asarray(inputs["i"], dtype=np.float32)
    t = np.asarray(inputs["t"], dtype=np.float32)
    weights = _host_pack(inputs)
    i16 = i.astype(np.float16)
    t16 = t.astype(np.float16)
    ntiles = BS // NT
    in_maps = []
    for c in range(NCORES):
        sl = slice(c * BS, (c + 1) * BS)
        m = dict(weights)
        # [ntiles, 128, XALL, NT]: batch-tile major; chunk 0 holds
        # t[n*NT+j, p], chunk 1+k holds i[n*NT+j, 128k+p].
        xi = i16[sl].T.reshape(KI, 128, ntiles, NT)   # [k, p, n, j]
        xt = t16[sl].T.reshape(TAB, ntiles, NT)       # [p, n, j]
        full = np.empty((ntiles, 128, XALL, NT), dtype=np.float16)
        full[:, :, 0, :] = xt.transpose(1, 0, 2)
        full[:, :, 1:, :] = xi.transpose(2, 1, 0, 3)
        m["iT"] = full
        in_maps.append(m)
    return in_maps


def kernel(**inputs) -> np.ndarray:
    from concourse import bass_utils

    if "nc" not in _CACHE:
        _CACHE["nc"] = _build_nc(BS)
    nc = _CACHE["nc"]

    in_maps = make_in_maps(inputs)
    res = bass_utils.run_bass_kernel_spmd(nc, in_maps, core_ids=list(range(NCORES)))

    out = np.empty((B, 2 * HID), dtype=np.float32)
    for c in range(NCORES):
        out[c * BS:(c + 1) * BS] = res.results[c]["outT"].T.astype(np.float32)
    return out

